# revision 36
# baseline (speedup 1.0000x reference)
"""Trainium2 Bass kernel for MultiHeadedAttentionBias.

Math (from the reference):
    v   = value @ W_v.T + b_v                      # [B,S,D] -> heads [B,H,S,dk]
    w   = where(mask==0, -1e9, bias)               # [B,H,S,S]
    p   = softmax(w, axis=-1)
    x   = einsum('bhqk,bhkd->bhqd', p, v)          # -> [B,S,D]
    out = x @ W_o.T + b_o
    return (out, bias)                             # bias passes through

query/key are unused by the reference.

Sharding: 8 cores = (batch b, query-half qh).  Each core handles all 8 heads
for 512 queries of one batch.  No collectives needed; host gathers.

Key layout/dtype choices:
  - The mask is folded into bias on the host (masked logits = -300, so
    exp underflows to exactly 0) and the per-core bias slice is shipped
    pre-transposed to [h, k, q] in bf16 -- fully contiguous DMA at half
    the bytes, and no mask traffic or on-chip mask multiply at all.
  - Scores stay transposed ([k, q], k on partitions), so the softmax
    denominator comes free from the PV matmul via an extra ones-column in
    the value tile (PSUM row 64), and x^T feeds the output projection
    directly as lhsT.  No on-chip transposes anywhere.
  - bf16 operands for the projections' inputs and the e/v path (single
    pass on the PE + FWL); the output projection runs in float32r
    (single-pass fp32-ish).  Accumulation is always f32 in PSUM.
  - Each head is processed in two half-chunks (DMA -> exp -> 4 PV
    matmuls) to keep the pipeline fine-grained and the PE HAM-warm.
  - All DMAs share one HWDGE ring (FIFO): weights/value first, bias
    halves next, output stores last -- each transfer gets full bandwidth
    and arrives in need-order.
"""

import sys

sys.path.insert(0, "/opt/trn_rl_repo")

import ml_dtypes
import numpy as np

import concourse.bass as bass
import concourse.mybir as mybir
import concourse.tile as tile
from concourse import bacc
from concourse.bass import ts
from concourse.bass_utils import run_bass_kernel_spmd

F32 = mybir.dt.float32
F32R = mybir.dt.float32r
BF16 = mybir.dt.bfloat16
EXP = mybir.ActivationFunctionType.Exp

NEG_FILL = -300.0          # masked logit; exp() underflows to 0

B, H, S, D = 4, 8, 1024, 512
DK = D // H                # 64
QC = 512                   # queries per core
N_CORES = 8
KC = S // 128              # 8 k-chunks of 128
KH = KC // 2               # k-chunks per half-head
TT = S // 128              # 8 token tiles for the value projection
JC = D // 128              # 4 feature chunks

_CACHE = {}

# Results of the last run_bass_kernel_spmd call (exec_time_ns etc. when
# tracing is enabled via BASS_TRACE=1); for use by test harnesses.
LAST_RESULTS = None


def _build_nc():
    nc = bacc.Bacc("TRN2", target_bir_lowering=False, debug=False,
                   num_devices=N_CORES)

    # all pre-arranged on the host to the exact SBUF tile layout, so every
    # DMA is one fully-contiguous linear transfer
    biasT = nc.dram_tensor("biasT", [H, 128, KC, QC], BF16,
                           kind="ExternalInput")
    valT = nc.dram_tensor("valT", [128, JC, S], BF16, kind="ExternalInput")
    wvT = nc.dram_tensor("wvT", [128, JC, D], BF16, kind="ExternalInput")
    woT = nc.dram_tensor("woT", [DK + 1, H, D], BF16, kind="ExternalInput")
    bv = nc.dram_tensor("bv", [1, D], F32, kind="ExternalInput")
    bo = nc.dram_tensor("bo", [1, D], F32, kind="ExternalInput")
    out_c = nc.dram_tensor("out_c", [QC, D], F32, kind="ExternalOutput")

    def bcast_ap(handle, parts):
        ap = handle[0, :]
        return bass.AP(tensor=ap.tensor, offset=ap.offset,
                       ap=[[0, parts]] + [list(d) for d in ap.ap])

    with tile.TileContext(nc) as tc:
        with (
            tc.tile_pool(name="singles", bufs=1) as singles,
            tc.tile_pool(name="ebin", bufs=10) as ebin_pool,
            tc.tile_pool(name="outs", bufs=4) as out_pool,
            tc.tile_pool(name="small", bufs=3) as small_pool,
            tc.tile_pool(name="mm128", bufs=4, space="PSUM") as mm128_pool,
            tc.tile_pool(name="px", bufs=4, space="PSUM") as px_pool,
        ):
            # ---- bias prefetch bookkeeping --------------------------
            # head 0 (and the last head) are split into halves so the exp
            # chain starts earlier at the pipeline head and the PV of the
            # last head starts earlier at the tail.  Each entry of
            # bias_tiles[h] is (tile, k0, nk): tile[:, 0:nk, :] covers
            # k-chunks k0..k0+nk-1.  All bias DMAs are pre-issued in
            # need-order with enough pool slots that none has a WAR dep;
            # heads 0/1 ride the gpsimd SWDGE ring, in parallel with the
            # weights on the sync HWDGE ring.
            bias_tiles = [None] * H

            def issue_bias(h):
                # halves only at the pipeline head (earlier first exp) and
                # tail (earlier last PV); middle heads use one full-head
                # DMA + exp -- each ACTIVATE costs a fixed 352 cycles, so
                # fewer, larger exps shave the ACT-bound chain
                if h in (0, H - 1):
                    parts = []
                    for half in range(2):
                        t = ebin_pool.tile([128, KH, QC], BF16, tag="ebin",
                                           name=f"ebin{h}_{half}")
                        nc.sync.dma_start(
                            out=t, in_=biasT[h, :, half * KH:(half + 1) * KH, :])
                        parts.append((t, half * KH, KH))
                    bias_tiles[h] = parts
                else:
                    t = ebin_pool.tile([128, KC, QC], BF16, tag="ebin",
                                       name=f"ebin{h}")
                    nc.sync.dma_start(out=t, in_=biasT[h])
                    bias_tiles[h] = [(t, 0, KC)]

            # bias for heads 0 and 1 first so the exp chain starts ASAP
            issue_bias(0)
            issue_bias(1)
            wv_sb = singles.tile([128, JC, D], BF16, tag="wv")
            nc.sync.dma_start(out=wv_sb, in_=wvT[:, :, :])
            vT_sb = singles.tile([128, JC, S], BF16, tag="vT")
            nc.sync.dma_start(out=vT_sb, in_=valT[:, :, :])
            bv_row = singles.tile([1, D], F32, tag="bv_row")
            nc.sync.dma_start(out=bv_row, in_=bv[:, :])
            bvb = singles.tile([128, D], F32, tag="bvb")
            nc.gpsimd.partition_broadcast(bvb, bv_row)
            wo_sb = singles.tile([DK + 1, H, D], BF16, tag="wo")

            v_aug = singles.tile([128, TT, H, DK + 1], BF16, tag="vaug")
            nc.vector.memset(v_aug[:, :, :, DK:DK + 1], 1.0)

            # ---- value projection: v = value @ W_v.T + b_v ----------
            # psum_v[t, j] = sum_k valT[k, t] * wvT[k, j]
            for tt in range(TT):
                psum_v = mm128_pool.tile([128, D], F32, tag="mm128")
                for kc in range(JC):
                    nc.tensor.matmul(psum_v,
                                     lhsT=vT_sb[:, kc, ts(tt, 128)],
                                     rhs=wv_sb[:, kc, :],
                                     start=(kc == 0), stop=(kc == JC - 1))
                nc.vector.tensor_add(
                    v_aug[:, tt, :, 0:DK],
                    psum_v[:].rearrange("p (h d) -> p h d", h=H),
                    bvb[:].rearrange("p (h d) -> p h d", h=H),
                )

            # ---- per-head: exp + PV matmul in two half-chunks -------
            psum_os = [None] * (QC // 128)
            xhs = [None] * H

            def emit_outproj(h):
                # accumulate head h's output-projection contribution; head
                # 0 carries an extra all-ones lhsT row that multiplies the
                # b_o row appended to wo, so the bias add comes for free
                nrow = DK + 1 if h == 0 else DK
                for tt in range(QC // 128):
                    if h == 0:
                        psum_o = mm128_pool.tile([128, D], F32, tag="mm128",
                                                 name=f"psum_o{tt}")
                        psum_os[tt] = psum_o
                    nc.tensor.matmul(psum_os[tt],
                                     lhsT=xhs[h][0:nrow, ts(tt, 128)],
                                     rhs=wo_sb[0:nrow, h, :],
                                     start=(h == 0), stop=(h == H - 1))
                    if h == H - 1:
                        # epilogue fused per-tt: copy out of PSUM on the
                        # (idle) ACT engine and store immediately
                        outt = out_pool.tile([128, D], F32, tag="outt",
                                             name=f"outt{tt}")
                        nc.scalar.copy(outt, psum_os[tt])
                        nc.sync.dma_start(out=out_c[ts(tt, 128), :], in_=outt)

            for h in range(H):
                if h + 2 < H:
                    issue_bias(h + 2)
                psum_x = px_pool.tile([DK + 1, QC], F32, tag="px")
                for ebin, k0, nk in bias_tiles[h]:
                    nc.scalar.activation(out=ebin, in_=ebin, func=EXP)
                    for kc in range(nk):
                        k = k0 + kc
                        nc.tensor.matmul(psum_x,
                                         lhsT=v_aug[:, k, h, :],
                                         rhs=ebin[:, kc, :],
                                         start=(k == 0), stop=(k == KC - 1))

                # psum_x rows 0..63 = x^T (unnormalized), row 64 = sums.
                # sums -> sbuf -> broadcast to 64 partitions (gpsimd) ->
                # reciprocal -> scale
                sums_sb = small_pool.tile([1, QC], F32, tag="sums")
                nc.vector.tensor_copy(sums_sb, psum_x[DK:DK + 1, :])
                rb = small_pool.tile([DK, QC], F32, tag="rb")
                nc.gpsimd.partition_broadcast(rb, sums_sb)
                rb2 = small_pool.tile([DK, QC], F32, tag="rb2")
                nc.vector.reciprocal_approx_fast(out=rb2, in_=rb)
                nrow = DK + 1 if h == 0 else DK
                xh = small_pool.tile([nrow, QC], BF16, tag="xh", bufs=4,
                                     name=f"xh{h}")
                xhs[h] = xh
                if h == 0:
                    nc.vector.memset(xh[DK:DK + 1, :], 1.0)
                nc.vector.tensor_mul(xh[0:DK, :], psum_x[0:DK, :], rb2)

                if h == 0:
                    # emitted here so it lands on the DMA ring after the
                    # early bias heads; must be emitted before the first
                    # emit_outproj so the RAW dep on wo_sb exists
                    nc.sync.dma_start(out=wo_sb, in_=woT[:, :, :])

                if h == H - 1:
                    # keep the PE warm (HAM) through the last head's
                    # normalize chain so the final out-proj matmuls run at
                    # full clock: filler matmuls on static data into a
                    # scratch PSUM tile, placed by queue order right after
                    # PV h7b and before the tail out-projs
                    scratch = px_pool.tile([DK, QC], F32, tag="px",
                                           name="warm_scratch")
                    for w in range(10):
                        nc.tensor.matmul(scratch,
                                         lhsT=wo_sb[0:1, 0, 0:DK],
                                         rhs=wo_sb[0:1, 1, :],
                                         start=True, stop=True)

                # emit out-proj two heads behind: with delay-1 the
                # in-order PE stalls ~1us per head on each fresh DVE mul
                # once the exp chain stops pacing (compounds over the back
                # half); two head-periods of slack absorb the chain latency
                if h >= 2:
                    emit_outproj(h - 2)
            emit_outproj(H - 2)
            emit_outproj(H - 1)



    nc.finalize()
    return nc


def kernel(query=None, key=None, value=None, bias=None, mask=None,
           W_v=None, b_v=None, W_o=None, b_o=None, **_unused):
    global LAST_RESULTS
    value = np.ascontiguousarray(np.asarray(value, dtype=np.float32))
    bias = np.asarray(bias, dtype=np.float32)
    mask = np.asarray(mask)
    W_v = np.asarray(W_v, dtype=np.float32)
    b_v = np.asarray(b_v, dtype=np.float32)
    W_o = np.asarray(W_o, dtype=np.float32)
    b_o = np.asarray(b_o, dtype=np.float32)

    if "nc" not in _CACHE:
        _CACHE["nc"] = _build_nc()
    nc = _CACHE["nc"]

    # wvT[p, c, j] = W_v.T[c*128+p, j];  woT[d, hh, o] = W_o.T[hh*64+d, o]
    wvT = np.ascontiguousarray(
        W_v.T.reshape(JC, 128, D).transpose(1, 0, 2)).astype(ml_dtypes.bfloat16)
    # woT_aug[d, hh, o] = W_o.T[hh*64+d, o]; row DK of head 0 carries b_o
    woT = np.zeros((DK + 1, H, D), dtype=np.float32)
    woT[:DK] = W_o.T.reshape(H, DK, D).transpose(1, 0, 2)
    woT[DK, 0, :] = b_o
    woT = np.ascontiguousarray(woT).astype(ml_dtypes.bfloat16)
    bv2 = np.ascontiguousarray(b_v.reshape(1, D))
    bo2 = np.ascontiguousarray(b_o.reshape(1, D))

    in_maps = []
    for c in range(N_CORES):
        b, qh = divmod(c, 2)
        q0 = qh * QC
        # fold the mask in (masked -> -300, exp() == 0), transpose to
        # [h, k, q], convert to bf16
        bias_slice = bias[b, :, q0:q0 + QC, :]          # [H, q, k]
        mask_slice = mask[b, q0:q0 + QC, :]             # [q, k]
        masked = np.where(mask_slice[None, :, :] == 0,
                          np.float32(NEG_FILL), bias_slice)
        # biasT[h, p, c, q] = masked[h, q, c*128 + p]
        biasT_c = np.ascontiguousarray(
            masked.transpose(0, 2, 1).reshape(H, KC, 128, QC)
            .transpose(0, 2, 1, 3)).astype(ml_dtypes.bfloat16)
        # valT[p, c, t] = value[b].T[c*128+p, t]
        valT_b = np.ascontiguousarray(
            value[b].T.reshape(JC, 128, S).transpose(1, 0, 2)
        ).astype(ml_dtypes.bfloat16)
        in_maps.append({
            "biasT": biasT_c,
            "valT": valT_b,
            "wvT": wvT,
            "woT": woT,
            "bv": bv2,
            "bo": bo2,
        })

    res = run_bass_kernel_spmd(nc, in_maps, core_ids=list(range(N_CORES)))
    LAST_RESULTS = res

    out = np.empty((B, S, D), dtype=np.float32)
    for c in range(N_CORES):
        b, qh = divmod(c, 2)
        q0 = qh * QC
        out[b, q0:q0 + QC, :] = res.results[c]["out_c"]
    return (out, bias)


# revision 37
# speedup vs baseline: 1.0344x; 1.0344x over previous
"""Trainium2 Bass kernel for MultiHeadedAttentionBias.

Math (from the reference):
    v   = value @ W_v.T + b_v                      # [B,S,D] -> heads [B,H,S,dk]
    w   = where(mask==0, -1e9, bias)               # [B,H,S,S]
    p   = softmax(w, axis=-1)
    x   = einsum('bhqk,bhkd->bhqd', p, v)          # -> [B,S,D]
    out = x @ W_o.T + b_o
    return (out, bias)                             # bias passes through

query/key are unused by the reference.

Sharding: 8 cores = (batch b, query-half qh).  Each core handles all 8 heads
for 512 queries of one batch.  No collectives needed; host gathers.

Key layout/dtype choices:
  - The mask is folded into bias on the host (masked logits = -300, so
    exp underflows to exactly 0) and the per-core bias slice is shipped
    pre-transposed to [h, k, q] in bf16 -- fully contiguous DMA at half
    the bytes, and no mask traffic or on-chip mask multiply at all.
  - Scores stay transposed ([k, q], k on partitions), so the softmax
    denominator comes free from the PV matmul via an extra ones-column in
    the value tile (PSUM row 64), and x^T feeds the output projection
    directly as lhsT.  No on-chip transposes anywhere.
  - bf16 operands for the projections' inputs and the e/v path (single
    pass on the PE + FWL); the output projection runs in float32r
    (single-pass fp32-ish).  Accumulation is always f32 in PSUM.
  - Each head is processed in two half-chunks (DMA -> exp -> 4 PV
    matmuls) to keep the pipeline fine-grained and the PE HAM-warm.
  - All DMAs share one HWDGE ring (FIFO): weights/value first, bias
    halves next, output stores last -- each transfer gets full bandwidth
    and arrives in need-order.
"""

import sys

sys.path.insert(0, "/opt/trn_rl_repo")

import ml_dtypes
import numpy as np

import concourse.bass as bass
import concourse.mybir as mybir
import concourse.tile as tile
from concourse import bacc
from concourse.bass import ts
from concourse.bass_utils import run_bass_kernel_spmd

F32 = mybir.dt.float32
F32R = mybir.dt.float32r
BF16 = mybir.dt.bfloat16
EXP = mybir.ActivationFunctionType.Exp

NEG_FILL = -300.0          # masked logit; exp() underflows to 0

B, H, S, D = 4, 8, 1024, 512
DK = D // H                # 64
QC = 512                   # queries per core
N_CORES = 8
KC = S // 128              # 8 k-chunks of 128
KH = KC // 2               # k-chunks per half-head
TT = S // 128              # 8 token tiles for the value projection
JC = D // 128              # 4 feature chunks

_CACHE = {}

# Results of the last run_bass_kernel_spmd call (exec_time_ns etc. when
# tracing is enabled via BASS_TRACE=1); for use by test harnesses.
LAST_RESULTS = None


def _build_nc():
    nc = bacc.Bacc("TRN2", target_bir_lowering=False, debug=False,
                   num_devices=N_CORES)

    # all pre-arranged on the host to the exact SBUF tile layout, so every
    # DMA is one fully-contiguous linear transfer
    biasT = nc.dram_tensor("biasT", [H, 128, KC, QC], BF16,
                           kind="ExternalInput")
    valT = nc.dram_tensor("valT", [128, JC, S], BF16, kind="ExternalInput")
    wvT = nc.dram_tensor("wvT", [128, JC, D], BF16, kind="ExternalInput")
    woT = nc.dram_tensor("woT", [DK + 1, H, D], BF16, kind="ExternalInput")
    bv = nc.dram_tensor("bv", [1, D], F32, kind="ExternalInput")
    bo = nc.dram_tensor("bo", [1, D], F32, kind="ExternalInput")
    out_c = nc.dram_tensor("out_c", [QC, D], F32, kind="ExternalOutput")

    def bcast_ap(handle, parts):
        ap = handle[0, :]
        return bass.AP(tensor=ap.tensor, offset=ap.offset,
                       ap=[[0, parts]] + [list(d) for d in ap.ap])

    with tile.TileContext(nc) as tc:
        with (
            tc.tile_pool(name="singles", bufs=1) as singles,
            tc.tile_pool(name="ebin", bufs=10) as ebin_pool,
            tc.tile_pool(name="outs", bufs=4) as out_pool,
            tc.tile_pool(name="small", bufs=3) as small_pool,
            tc.tile_pool(name="mm128", bufs=4, space="PSUM") as mm128_pool,
            tc.tile_pool(name="px", bufs=4, space="PSUM") as px_pool,
        ):
            # ---- bias prefetch bookkeeping --------------------------
            # head 0 (and the last head) are split into halves so the exp
            # chain starts earlier at the pipeline head and the PV of the
            # last head starts earlier at the tail.  Each entry of
            # bias_tiles[h] is (tile, k0, nk): tile[:, 0:nk, :] covers
            # k-chunks k0..k0+nk-1.  All bias DMAs are pre-issued in
            # need-order with enough pool slots that none has a WAR dep;
            # heads 0/1 ride the gpsimd SWDGE ring, in parallel with the
            # weights on the sync HWDGE ring.
            bias_tiles = [None] * H

            def issue_bias(h):
                # halves only at the pipeline head (earlier first exp) and
                # tail (earlier last PV); middle heads use one full-head
                # DMA + exp -- each ACTIVATE costs a fixed 352 cycles, so
                # fewer, larger exps shave the ACT-bound chain
                if h in (0, H - 1):
                    parts = []
                    for half in range(2):
                        t = ebin_pool.tile([128, KH, QC], BF16, tag="ebin",
                                           name=f"ebin{h}_{half}")
                        nc.sync.dma_start(
                            out=t, in_=biasT[h, :, half * KH:(half + 1) * KH, :])
                        parts.append((t, half * KH, KH))
                    bias_tiles[h] = parts
                else:
                    t = ebin_pool.tile([128, KC, QC], BF16, tag="ebin",
                                       name=f"ebin{h}")
                    nc.sync.dma_start(out=t, in_=biasT[h])
                    bias_tiles[h] = [(t, 0, KC)]

            # bias for heads 0 and 1 first so the exp chain starts ASAP
            issue_bias(0)
            issue_bias(1)
            wv_sb = singles.tile([128, JC, D], BF16, tag="wv")
            nc.sync.dma_start(out=wv_sb, in_=wvT[:, :, :])
            vT_sb = singles.tile([128, JC, S], BF16, tag="vT")
            nc.sync.dma_start(out=vT_sb, in_=valT[:, :, :])
            bv_row = singles.tile([1, D], F32, tag="bv_row")
            nc.sync.dma_start(out=bv_row, in_=bv[:, :])
            bvb = singles.tile([128, D], F32, tag="bvb")
            nc.gpsimd.partition_broadcast(bvb, bv_row)
            wo_sb = singles.tile([DK + 1, H, D], BF16, tag="wo")

            v_aug = singles.tile([128, TT, H, DK + 1], BF16, tag="vaug")
            nc.vector.memset(v_aug[:, :, :, DK:DK + 1], 1.0)

            # ---- value projection: v = value @ W_v.T + b_v ----------
            # psum_v[t, j] = sum_k valT[k, t] * wvT[k, j]
            for tt in range(TT):
                psum_v = mm128_pool.tile([128, D], F32, tag="mm128")
                for kc in range(JC):
                    nc.tensor.matmul(psum_v,
                                     lhsT=vT_sb[:, kc, ts(tt, 128)],
                                     rhs=wv_sb[:, kc, :],
                                     start=(kc == 0), stop=(kc == JC - 1))
                nc.vector.tensor_add(
                    v_aug[:, tt, :, 0:DK],
                    psum_v[:].rearrange("p (h d) -> p h d", h=H),
                    bvb[:].rearrange("p (h d) -> p h d", h=H),
                )

            # ---- per-head: exp + PV matmul in two half-chunks -------
            psum_os = [None] * (QC // 128)
            xhs = [None] * H

            def emit_outproj(h):
                # accumulate head h's output-projection contribution; head
                # 0 carries an extra all-ones lhsT row that multiplies the
                # b_o row appended to wo, so the bias add comes for free
                nrow = DK + 1 if h == 0 else DK
                for tt in range(QC // 128):
                    if h == 0:
                        psum_o = mm128_pool.tile([128, D], F32, tag="mm128",
                                                 name=f"psum_o{tt}")
                        psum_os[tt] = psum_o
                    nc.tensor.matmul(psum_os[tt],
                                     lhsT=xhs[h][0:nrow, ts(tt, 128)],
                                     rhs=wo_sb[0:nrow, h, :],
                                     start=(h == 0), stop=(h == H - 1))
                    if h == H - 1:
                        # epilogue fused per-tt: copy out of PSUM, stores
                        # issued immediately; copies alternate between the
                        # (idle-at-tail) ACT and DVE engines so the four
                        # copies run two-deep in parallel
                        outt = out_pool.tile([128, D], F32, tag="outt",
                                             name=f"outt{tt}")
                        if tt % 2 == 0:
                            nc.scalar.copy(outt, psum_os[tt])
                        else:
                            nc.vector.tensor_copy(outt, psum_os[tt])
                        nc.sync.dma_start(out=out_c[ts(tt, 128), :], in_=outt)

            for h in range(H):
                if h + 2 < H:
                    issue_bias(h + 2)
                psum_x = px_pool.tile([DK + 1, QC], F32, tag="px")
                for ebin, k0, nk in bias_tiles[h]:
                    nc.scalar.activation(out=ebin, in_=ebin, func=EXP)
                    for kc in range(nk):
                        k = k0 + kc
                        nc.tensor.matmul(psum_x,
                                         lhsT=v_aug[:, k, h, :],
                                         rhs=ebin[:, kc, :],
                                         start=(k == 0), stop=(k == KC - 1))

                # psum_x rows 0..63 = x^T (unnormalized), row 64 = sums.
                # sums -> sbuf -> broadcast to 64 partitions (gpsimd) ->
                # reciprocal -> scale
                sums_sb = small_pool.tile([1, QC], F32, tag="sums")
                nc.vector.tensor_copy(sums_sb, psum_x[DK:DK + 1, :])
                rb = small_pool.tile([DK, QC], F32, tag="rb")
                nc.gpsimd.partition_broadcast(rb, sums_sb)
                rb2 = small_pool.tile([DK, QC], F32, tag="rb2")
                nc.vector.reciprocal_approx_fast(out=rb2, in_=rb)
                nrow = DK + 1 if h == 0 else DK
                xh = small_pool.tile([nrow, QC], BF16, tag="xh", bufs=4,
                                     name=f"xh{h}")
                xhs[h] = xh
                if h == 0:
                    nc.vector.memset(xh[DK:DK + 1, :], 1.0)
                nc.vector.tensor_mul(xh[0:DK, :], psum_x[0:DK, :], rb2)

                if h == 0:
                    # emitted here so it lands on the DMA ring after the
                    # early bias heads; must be emitted before the first
                    # emit_outproj so the RAW dep on wo_sb exists
                    nc.sync.dma_start(out=wo_sb, in_=woT[:, :, :])

                if h == H - 1:
                    # keep the PE warm (HAM) through the last head's
                    # normalize chain so the final out-proj matmuls run at
                    # full clock: filler matmuls on static data into a
                    # scratch PSUM tile, placed by queue order right after
                    # PV h7b and before the tail out-projs
                    scratch = px_pool.tile([DK, QC], F32, tag="px",
                                           name="warm_scratch")
                    for w in range(10):
                        nc.tensor.matmul(scratch,
                                         lhsT=wo_sb[0:1, 0, 0:DK],
                                         rhs=wo_sb[0:1, 1, :],
                                         start=True, stop=True)

                # emit out-proj two heads behind: with delay-1 the
                # in-order PE stalls ~1us per head on each fresh DVE mul
                # once the exp chain stops pacing (compounds over the back
                # half); two head-periods of slack absorb the chain latency
                if h >= 2:
                    emit_outproj(h - 2)
            emit_outproj(H - 2)
            emit_outproj(H - 1)



    nc.finalize()
    return nc


def kernel(query=None, key=None, value=None, bias=None, mask=None,
           W_v=None, b_v=None, W_o=None, b_o=None, **_unused):
    global LAST_RESULTS
    value = np.ascontiguousarray(np.asarray(value, dtype=np.float32))
    bias = np.asarray(bias, dtype=np.float32)
    mask = np.asarray(mask)
    W_v = np.asarray(W_v, dtype=np.float32)
    b_v = np.asarray(b_v, dtype=np.float32)
    W_o = np.asarray(W_o, dtype=np.float32)
    b_o = np.asarray(b_o, dtype=np.float32)

    if "nc" not in _CACHE:
        _CACHE["nc"] = _build_nc()
    nc = _CACHE["nc"]

    # wvT[p, c, j] = W_v.T[c*128+p, j];  woT[d, hh, o] = W_o.T[hh*64+d, o]
    wvT = np.ascontiguousarray(
        W_v.T.reshape(JC, 128, D).transpose(1, 0, 2)).astype(ml_dtypes.bfloat16)
    # woT_aug[d, hh, o] = W_o.T[hh*64+d, o]; row DK of head 0 carries b_o
    woT = np.zeros((DK + 1, H, D), dtype=np.float32)
    woT[:DK] = W_o.T.reshape(H, DK, D).transpose(1, 0, 2)
    woT[DK, 0, :] = b_o
    woT = np.ascontiguousarray(woT).astype(ml_dtypes.bfloat16)
    bv2 = np.ascontiguousarray(b_v.reshape(1, D))
    bo2 = np.ascontiguousarray(b_o.reshape(1, D))

    in_maps = []
    for c in range(N_CORES):
        b, qh = divmod(c, 2)
        q0 = qh * QC
        # fold the mask in (masked -> -300, exp() == 0), transpose to
        # [h, k, q], convert to bf16
        bias_slice = bias[b, :, q0:q0 + QC, :]          # [H, q, k]
        mask_slice = mask[b, q0:q0 + QC, :]             # [q, k]
        masked = np.where(mask_slice[None, :, :] == 0,
                          np.float32(NEG_FILL), bias_slice)
        # biasT[h, p, c, q] = masked[h, q, c*128 + p]
        biasT_c = np.ascontiguousarray(
            masked.transpose(0, 2, 1).reshape(H, KC, 128, QC)
            .transpose(0, 2, 1, 3)).astype(ml_dtypes.bfloat16)
        # valT[p, c, t] = value[b].T[c*128+p, t]
        valT_b = np.ascontiguousarray(
            value[b].T.reshape(JC, 128, S).transpose(1, 0, 2)
        ).astype(ml_dtypes.bfloat16)
        in_maps.append({
            "biasT": biasT_c,
            "valT": valT_b,
            "wvT": wvT,
            "woT": woT,
            "bv": bv2,
            "bo": bo2,
        })

    res = run_bass_kernel_spmd(nc, in_maps, core_ids=list(range(N_CORES)))
    LAST_RESULTS = res

    out = np.empty((B, S, D), dtype=np.float32)
    for c in range(N_CORES):
        b, qh = divmod(c, 2)
        q0 = qh * QC
        out[b, q0:q0 + QC, :] = res.results[c]["out_c"]
    return (out, bias)
